# revision 17
# baseline (speedup 1.0000x reference)
"""Causal self-attention (B=8, T=1024, C=768, H=12) on 8 Trainium2 NeuronCores.

Sharding: data parallel — one batch element per core, no collectives.

Per-core Bass/Tile kernel, all matmul operands bf16 (1 cyc/row at any N;
fp32 PSUM accumulate; host pre-quantizes weights + x to bf16):
  Q^T, K^T = Wqkv.T @ x^T                  (features on partitions)
  V = x^T.T @ Wv                           (natural layout, per-head + ones col)
  per head: S^T = K_h^T.T @ Q_h^T          (k on partitions, exact causal chunks)
    causal mask on the diagonal 128x128 block added on PSUM via a bf16
    matmul (negmask^T.T @ I), then ACT exp(s/8) -> ragged P^T
    O'^T = [V_h | 1].T @ P^T               (row 64 = softmax denominator)
    normalize: DVE reciprocal + tiny DMA to partition 0 + GPSIMD
    partition_broadcast + DVE multiply (odd heads take an SBUF->SBUF DMA to
    reach partitions 64..127 — DVE lanes are partition-locked)
  y = attn'^T.T @ Wp + bias
"""
import sys
from contextlib import ExitStack

import numpy as np

for _p in ("/opt/trn_rl_repo", "/root/.axon_site/_ro/trn_rl_repo"):
    if _p not in sys.path:
        sys.path.insert(0, _p)

import concourse.bass as bass  # noqa: E402
import concourse.mybir as mybir  # noqa: E402

F32 = mybir.dt.float32
BF16 = mybir.dt.bfloat16
AF = mybir.ActivationFunctionType
OP = mybir.AluOpType

B, T, C, H, D = 8, 1024, 768, 12, 64
N_CORES = 8


def _chunks_512(a, b):
    out = []
    while a < b:
        nxt = min((a // 512 + 1) * 512, b)
        out.append((a, nxt))
        a = nxt
    return out


def _emit_attention(tc, io):
    nc = tc.nc
    NT = T // 128
    NC = C // 128

    with ExitStack() as stack:
        persist = stack.enter_context(tc.tile_pool(name="persist", bufs=1))
        consts = stack.enter_context(tc.tile_pool(name="consts", bufs=1))
        negmT = consts.tile([128, 128], BF16, tag="negmT")
        idb = consts.tile([128, 128], BF16, tag="idb")
        nc.sync.dma_start(negmT[:], io["negmaskT16"])
        nc.sync.dma_start(idb[:], io["identity16"])
        bqt_sb = consts.tile([128, 3 * C // 128], F32, tag="bqt")
        nc.sync.dma_start(bqt_sb[:], io["bqkvT"])
        bb_sb = consts.tile([128, 2 * C], F32, tag="bb")
        nc.sync.dma_start(bb_sb[:], io["bias_bcast"])
        ones_sb = consts.tile([128, 128], F32, tag="ones_sb")
        nc.sync.dma_start(ones_sb[:], io["ones"])

        qt = persist.tile([128, NC, T], BF16, tag="qt")
        kt_ = persist.tile([128, NC, T], BF16, tag="kt")
        vp = persist.tile([128, NT, H, D + 1], BF16, tag="vp")

        p23 = stack.enter_context(tc.tile_pool(name="p23", bufs=1))
        attnT = p23.tile([128, NC, T], BF16, tag="attnT")
        wpp = p23.tile([128, NC, C], BF16, tag="wpp")

        # ---------------- phase 1: x^T, Q^T, K^T, V ----------------
        with tc.tile_pool(name="p1w", bufs=1) as p1w, \
             tc.tile_pool(name="ps1b", bufs=3, space="PSUM") as ps1b:
            x1t = p1w.tile([128, NC, T], BF16, tag="x1t")
            wq_sb = p1w.tile([128, NC, 3 * C], BF16, tag="wq")
            nc.vector.tensor_copy(
                vp[:, :, :, 64],
                ones_sb[:, 0:NT * H].rearrange("p (t h) -> p t h", h=H))

            # DMA order = PE consumption order: the first QK weight group,
            # then x (the first matmul contracts over all of x), then the
            # remaining weight groups (256-col, m_order-interleaved), V, wp.
            wq_groups = []
            for mp in range(NC // 2):
                wq_groups.append((mp * 256, mp * 256 + 256))
                wq_groups.append((C + mp * 256, C + mp * 256 + 256))
            wq_groups += _chunks_512(2 * C, 3 * C)

            def load_wq(w0, w1):
                for kt in range(NC):
                    nc.sync.dma_start(
                        wq_sb[:, kt, w0:w1],
                        io["wqkv"][kt * 128:(kt + 1) * 128, w0:w1])

            load_wq(*wq_groups[0])
            for c in range(NC):
                nc.sync.dma_start(x1t[:, c, :],
                                  io["xT"][c * 128:(c + 1) * 128, :])
            for (w0, w1) in wq_groups[1:]:
                load_wq(w0, w1)
            for kt in range(NC):
                nc.sync.dma_start(wpp[:, kt, :],
                                  io["wp"][kt * 128:(kt + 1) * 128, :])

            m_order = [ft + o for ft in range(NC) for o in (0, NC)]
            for m in m_order:
                dest = qt if m < NC else kt_
                mm = m % NC
                ps = ps1b.tile([128, T], F32, tag="ps_mm")
                for (a, b) in _chunks_512(0, T):
                    for kt in range(NC):
                        nc.tensor.matmul(
                            ps[:, a:b], wq_sb[:, kt, m * 128:(m + 1) * 128],
                            x1t[:, kt, a:b], start=(kt == 0),
                            stop=(kt == NC - 1))
                # bias is per-partition here: fuse it into the copy
                nc.vector.tensor_scalar_add(dest[:, mm, :], ps[:],
                                            bqt_sb[:, m:m + 1])

            for t in range(NT):
                ps = ps1b.tile([128, T], F32, tag="ps_mm")
                for (n0, n1) in _chunks_512(0, C):
                    for kt in range(NC):
                        nc.tensor.matmul(
                            ps[:, n0:n1], x1t[:, kt, t * 128:(t + 1) * 128],
                            wq_sb[:, kt, 2 * C + n0:2 * C + n1],
                            start=(kt == 0), stop=(kt == NC - 1))
                nc.vector.tensor_tensor(
                    vp[:, t, :, 0:D],
                    ps[:, 0:C].rearrange("p (h d) -> p h d", d=D),
                    bb_sb[:, 0:C].rearrange("p (h d) -> p h d", d=D),
                    OP.add)

        # ---------------- phase 2: attention ----------------
        off = [0] * (NT + 1)
        for i in range(NT):
            off[i + 1] = off[i] + (T - 128 * i)
        PTW = off[NT]

        with tc.tile_pool(name="p2", bufs=2) as p2, \
             tc.tile_pool(name="p2o", bufs=3) as p2o, \
             tc.tile_pool(name="ps2a", bufs=3, space="PSUM") as ps2a, \
             tc.tile_pool(name="ps2b", bufs=2, space="PSUM") as ps2b:
            head_order = [hp * 2 + o for hp in range(H // 2) for o in (1, 0)]
            for h in head_order:
                p0 = 64 * (h % 2)      # partition base of this head's features
                ft = h // 2            # feature tile
                pt_sb = p2.tile([128, PTW], BF16, tag="pt", name=f"pt{h}")
                for kt in range(NT):
                    base = (kt * 128 // 512) * 512
                    ps_s = ps2a.tile([128, T - base], F32, tag="ps_s")
                    for (a, b) in _chunks_512(kt * 128, T):
                        diag = a == kt * 128
                        nc.tensor.matmul(
                            ps_s[:, a - base:b - base],
                            kt_[p0:p0 + 64, ft, kt * 128:(kt + 1) * 128],
                            qt[p0:p0 + 64, ft, a:b],
                            start=True, stop=not diag)
                        if diag:
                            nc.tensor.matmul(ps_s[:, a - base:a - base + 128],
                                             negmT[:], idb[:],
                                             start=False, stop=True)
                    # one exp per strip -> ragged P^T
                    nc.scalar.activation(
                        pt_sb[:, off[kt]:off[kt + 1]],
                        ps_s[:, kt * 128 - base:T - base],
                        AF.Exp, bias=0.0, scale=1.0 / np.sqrt(D))
                for (q0, q1) in _chunks_512(0, T):
                    kt_max = q1 // 128
                    ps_o = ps2b.tile([65, 512], F32, tag="ps_o")
                    for kt in range(kt_max):
                        a = max(q0, kt * 128)
                        rhs = pt_sb[:, off[kt] + a - kt * 128:
                                    off[kt] + q1 - kt * 128]
                        nc.tensor.matmul(
                            ps_o[:, a - q0:q1 - q0],
                            vp[:, kt, h, :], rhs,
                            start=(kt == 0), stop=(kt == kt_max - 1))
                    w = q1 - q0
                    dn = p2o.tile([65, 512], F32, tag="dn")
                    nc.vector.reciprocal(dn[64:65, 0:w], ps_o[64:65, 0:w])
                    bc = p2o.tile([128, 512], F32, tag="bc")
                    # partition_broadcast reads partition 0 only:
                    # stage the denominator row there via a tiny DMA
                    dn0 = p2o.tile([1, 512], F32, tag="dn0")
                    nc.sync.dma_start(dn0[0:1, 0:w], dn[64:65, 0:w])
                    nc.gpsimd.partition_broadcast(bc[:, 0:w], dn0[0:1, 0:w])
                    if h % 2 == 0:
                        nc.vector.tensor_tensor(
                            attnT[0:64, ft, q0:q1],
                            ps_o[0:64, 0:w], bc[0:64, 0:w], OP.mult)
                    else:
                        o_n = p2o.tile([64, 512], BF16, tag="o_n")
                        nc.vector.tensor_tensor(
                            o_n[:, 0:w], ps_o[0:64, 0:w],
                            bc[0:64, 0:w], OP.mult)
                        nc.sync.dma_start(
                            attnT[64:128, ft, q0:q1], o_n[:, 0:w])

        # ---------------- phase 3: projection ----------------
        with tc.tile_pool(name="p3", bufs=3) as p3, \
             tc.tile_pool(name="ps3", bufs=3, space="PSUM") as ps3:
            for t in range(NT):
                ps_y = ps3.tile([128, C], F32, tag="ps_y")
                for (n0, n1) in _chunks_512(0, C):
                    for kt in range(NC):
                        nc.tensor.matmul(
                            ps_y[:, n0:n1], attnT[:, kt, t * 128:(t + 1) * 128],
                            wpp[:, kt, n0:n1],
                            start=(kt == 0), stop=(kt == NC - 1))
                y_sb = p3.tile([128, C], F32, tag="y_sb")
                nc.vector.tensor_tensor(y_sb[:], ps_y[:],
                                        bb_sb[:, C:2 * C], OP.add)
                nc.sync.dma_start(io["y"][t * 128:(t + 1) * 128, :], y_sb[:])


IO_SPECS = {
    "xT": ([C, T], BF16),
    "wqkv": ([C, 3 * C], BF16),
    "bqkvT": ([128, 3 * C // 128], F32),
    "bias_bcast": ([128, 2 * C], F32),
    "wp": ([C, C], BF16),
    "ones": ([128, 128], F32),
    "negmaskT16": ([128, 128], BF16),
    "identity16": ([128, 128], BF16),
}
OUT_SPECS = {"y": ([T, C], F32)}


def build_nc():
    from concourse import bacc
    import concourse.tile as tile
    nc = bacc.Bacc("TRN2", target_bir_lowering=False, debug=False,
                   enable_asserts=True, num_devices=N_CORES)
    io = {}
    for name, (shape, dt) in IO_SPECS.items():
        io[name] = nc.dram_tensor(name, shape, dt, kind="ExternalInput").ap()
    for name, (shape, dt) in OUT_SPECS.items():
        io[name] = nc.dram_tensor(name, shape, dt, kind="ExternalOutput").ap()
    with tile.TileContext(nc) as tc:
        _emit_attention(tc, io)
    nc.compile()
    return nc


def host_consts():
    import ml_dtypes
    negmask = np.where(np.triu(np.ones((128, 128), dtype=bool)), 0.0,
                       -1e9).astype(np.float32)
    return {
        "ones": np.ones((128, 128), dtype=np.float32),
        "negmaskT16": np.ascontiguousarray(negmask.T).astype(ml_dtypes.bfloat16),
        "identity16": np.eye(128, dtype=ml_dtypes.bfloat16),
    }


_NC_CACHE = None


def _get_nc():
    global _NC_CACHE
    if _NC_CACHE is None:
        _NC_CACHE = build_nc()
    return _NC_CACHE


def make_in_maps(x, c_attn_kernel, c_attn_bias, c_proj_kernel, c_proj_bias):
    import ml_dtypes
    BF = ml_dtypes.bfloat16
    consts = host_consts()
    wqkv = np.ascontiguousarray(c_attn_kernel).astype(BF)
    bqkv = np.ascontiguousarray(c_attn_bias, dtype=np.float32)
    bqkvT = np.ascontiguousarray(bqkv.reshape(3 * C // 128, 128).T)
    wp = np.ascontiguousarray(c_proj_kernel).astype(BF)
    bp = np.ascontiguousarray(c_proj_bias, dtype=np.float32)
    bias_bcast = np.ascontiguousarray(
        np.tile(np.concatenate([bqkv[2 * C:], bp]), (128, 1)))
    in_maps = []
    for bb in range(N_CORES):
        m = {"xT": np.ascontiguousarray(np.asarray(x[bb]).T).astype(BF),
             "wqkv": wqkv, "bqkvT": bqkvT, "wp": wp,
             "bias_bcast": bias_bcast}
        m.update(consts)
        in_maps.append(m)
    return in_maps


def kernel(x, c_attn_kernel, c_attn_bias, c_proj_kernel, c_proj_bias):
    from concourse.bass_utils import run_bass_kernel_spmd
    x = np.asarray(x)
    assert x.shape == (B, T, C), x.shape
    nc = _get_nc()
    in_maps = make_in_maps(x, c_attn_kernel, c_attn_bias, c_proj_kernel,
                           c_proj_bias)
    res = run_bass_kernel_spmd(nc, in_maps, core_ids=list(range(N_CORES)))
    y = np.stack([res.results[bb]["y"] for bb in range(N_CORES)]).astype(np.float32)
    return y


# revision 25
# speedup vs baseline: 1.1570x; 1.1570x over previous
"""Causal self-attention (B=8, T=1024, C=768, H=12) on 8 Trainium2 NeuronCores.

Sharding: data parallel — one batch element per core, no collectives.

Per-core Bass/Tile kernel, all matmul operands bf16 (1 cyc/row at any N;
fp32 PSUM accumulate; host pre-quantizes weights + x to bf16):
  Q^T, K^T = Wqkv.T @ x^T                  (features on partitions)
  V = x^T.T @ Wv                           (natural layout, per-head + ones col)
  per head: S^T = K_h^T.T @ Q_h^T          (k on partitions, exact causal chunks)
    causal mask on the diagonal 128x128 block added on PSUM via a bf16
    matmul (negmask^T.T @ I), then ACT exp(s/8) -> ragged P^T
    O'^T = [V_h | 1].T @ P^T               (row 64 = softmax denominator)
    normalize: DVE reciprocal + tiny DMA to partition 0 + GPSIMD
    partition_broadcast + DVE multiply (odd heads take an SBUF->SBUF DMA to
    reach partitions 64..127 — DVE lanes are partition-locked)
  y = attn'^T.T @ Wp + bias
"""
import sys
from contextlib import ExitStack

import numpy as np

for _p in ("/opt/trn_rl_repo", "/root/.axon_site/_ro/trn_rl_repo"):
    if _p not in sys.path:
        sys.path.insert(0, _p)

import concourse.bass as bass  # noqa: E402
import concourse.mybir as mybir  # noqa: E402

F32 = mybir.dt.float32
BF16 = mybir.dt.bfloat16
AF = mybir.ActivationFunctionType
OP = mybir.AluOpType

B, T, C, H, D = 8, 1024, 768, 12, 64
N_CORES = 8


def _chunks_512(a, b):
    out = []
    while a < b:
        nxt = min((a // 512 + 1) * 512, b)
        out.append((a, nxt))
        a = nxt
    return out


def _emit_attention(tc, io):
    nc = tc.nc
    NT = T // 128
    NC = C // 128

    with ExitStack() as stack:
        persist = stack.enter_context(tc.tile_pool(name="persist", bufs=1))
        consts = stack.enter_context(tc.tile_pool(name="consts", bufs=1))
        negmT = consts.tile([128, 128], BF16, tag="negmT")
        idb = consts.tile([128, 128], BF16, tag="idb")
        nc.sync.dma_start(negmT[:], io["negmaskT16"])
        nc.sync.dma_start(idb[:], io["identity16"])
        bqt_sb = consts.tile([128, 3 * C // 128], F32, tag="bqt")
        nc.sync.dma_start(bqt_sb[:], io["bqkvT"])
        bb_sb = consts.tile([128, 2 * C], F32, tag="bb")
        nc.sync.dma_start(bb_sb[:], io["bias_bcast"])
        ones_sb = consts.tile([128, 128], F32, tag="ones_sb")
        nc.sync.dma_start(ones_sb[:], io["ones"])

        qt = persist.tile([128, NC, T], BF16, tag="qt")
        kt_ = persist.tile([128, NC, T], BF16, tag="kt")
        vp = persist.tile([128, NT, H, D + 1], BF16, tag="vp")

        p23 = stack.enter_context(tc.tile_pool(name="p23", bufs=1))
        attnT = p23.tile([128, NC, T], BF16, tag="attnT")
        wpp = p23.tile([128, NC, C], BF16, tag="wpp")

        # ---------------- phase 1: x^T, Q^T, K^T, V ----------------
        with tc.tile_pool(name="p1w", bufs=1) as p1w, \
             tc.tile_pool(name="ps1b", bufs=3, space="PSUM") as ps1b:
            x1t = p1w.tile([128, NC, T], BF16, tag="x1t")
            wq_sb = p1w.tile([128, NC, 3 * C], BF16, tag="wq")
            nc.vector.tensor_copy(
                vp[:, :, :, 64],
                ones_sb[:, 0:NT * H].rearrange("p (t h) -> p t h", h=H))

            # DMA order = PE consumption order: the first QK weight group,
            # then x (the first matmul contracts over all of x), then the
            # remaining weight groups (256-col, m_order-interleaved), V, wp.
            wq_groups = []
            for mp in range(NC // 2):
                wq_groups.append((mp * 256, mp * 256 + 256))
                wq_groups.append((C + mp * 256, C + mp * 256 + 256))
            wq_groups += _chunks_512(2 * C, 3 * C)

            def load_wq(w0, w1):
                for kt in range(NC):
                    nc.sync.dma_start(
                        wq_sb[:, kt, w0:w1],
                        io["wqkv"][kt * 128:(kt + 1) * 128, w0:w1])

            load_wq(*wq_groups[0])
            for c in range(NC):
                nc.sync.dma_start(x1t[:, c, :],
                                  io["xT"][c * 128:(c + 1) * 128, :])
            for (w0, w1) in wq_groups[1:]:
                load_wq(w0, w1)
            for kt in range(NC):
                nc.sync.dma_start(wpp[:, kt, :],
                                  io["wp"][kt * 128:(kt + 1) * 128, :])

            m_order = [ft + o for ft in range(NC) for o in (0, NC)]
            for m in m_order:
                dest = qt if m < NC else kt_
                mm = m % NC
                ps = ps1b.tile([128, T], F32, tag="ps_mm")
                for (a, b) in _chunks_512(0, T):
                    for kt in range(NC):
                        nc.tensor.matmul(
                            ps[:, a:b], wq_sb[:, kt, m * 128:(m + 1) * 128],
                            x1t[:, kt, a:b], start=(kt == 0),
                            stop=(kt == NC - 1))
                # bias is per-partition here: fuse it into the copy
                nc.vector.tensor_scalar_add(dest[:, mm, :], ps[:],
                                            bqt_sb[:, m:m + 1])

            for t in range(NT):
                ps = ps1b.tile([128, T], F32, tag="ps_mm")
                for (n0, n1) in _chunks_512(0, C):
                    for kt in range(NC):
                        nc.tensor.matmul(
                            ps[:, n0:n1], x1t[:, kt, t * 128:(t + 1) * 128],
                            wq_sb[:, kt, 2 * C + n0:2 * C + n1],
                            start=(kt == 0), stop=(kt == NC - 1))
                nc.vector.tensor_tensor(
                    vp[:, t, :, 0:D],
                    ps[:, 0:C].rearrange("p (h d) -> p h d", d=D),
                    bb_sb[:, 0:C].rearrange("p (h d) -> p h d", d=D),
                    OP.add)

        # ---------------- phase 2: attention ----------------
        off = [0] * (NT + 1)
        for i in range(NT):
            off[i + 1] = off[i] + (T - 128 * i)
        PTW = off[NT]

        with tc.tile_pool(name="p2", bufs=2) as p2, \
             tc.tile_pool(name="p2o", bufs=3) as p2o, \
             tc.tile_pool(name="p2p", bufs=2) as p2p, \
             tc.tile_pool(name="ps2a", bufs=3, space="PSUM") as ps2a, \
             tc.tile_pool(name="ps2b", bufs=2, space="PSUM") as ps2b:
            head_order = [hp * 2 + o for hp in range(H // 2) for o in (1, 0)]
            onrm = None
            pending_tr = None

            def flush_tr():
                nonlocal pending_tr
                if pending_tr is None:
                    return
                onrm_p, ft_p = pending_tr
                pending_tr = None
                # transpose [q, (pair, d)] -> [(pair, d), q] feature-major
                ps_t = ps2a.tile([128, T], F32, tag="ps_s")
                ps_tb = ps_t.bitcast(BF16)
                for qt in range(NT):
                    nc.tensor.transpose(ps_tb[:, qt * 128:(qt + 1) * 128],
                                        onrm_p[:, qt, :, :], idb[:])
                nc.vector.tensor_copy(attnT[:, ft_p, :], ps_tb[:, 0:T])

            for h in head_order:
                p0 = 64 * (h % 2)      # partition base of this head's features
                ft = h // 2            # feature tile
                pt_sb = p2.tile([128, PTW], BF16, tag="pt", name=f"pt{h}")
                for kt in range(NT):
                    base = (kt * 128 // 512) * 512
                    ps_s = ps2a.tile([128, T - base], F32, tag="ps_s")
                    for (a, b) in _chunks_512(kt * 128, T):
                        diag = a == kt * 128
                        nc.tensor.matmul(
                            ps_s[:, a - base:b - base],
                            kt_[p0:p0 + 64, ft, kt * 128:(kt + 1) * 128],
                            qt[p0:p0 + 64, ft, a:b],
                            start=True, stop=not diag)
                        if diag:
                            nc.tensor.matmul(ps_s[:, a - base:a - base + 128],
                                             negmT[:], idb[:],
                                             start=False, stop=True)
                    # one exp per strip -> ragged P^T
                    nc.scalar.activation(
                        pt_sb[:, off[kt]:off[kt + 1]],
                        ps_s[:, kt * 128 - base:T - base],
                        AF.Exp, bias=0.0, scale=1.0 / np.sqrt(D))
                if h % 2 == 1:
                    # first head of the feature-tile pair; previous pair's
                    # transposes flush here so they don't stall on its DVE
                    flush_tr()
                    onrm = p2p.tile([128, NT, 2, D], BF16, tag="onrm")
                hi = h % 2  # pair row: even head -> 0, odd -> 1
                for (q0, q1) in _chunks_512(0, T):
                    nq = (q1 - q0) // 128
                    qb = q0 // 128
                    # O = P^T.T @ [V|1]: q on partitions -> per-partition
                    # denominator in column D, normalize with a DVE
                    # broadcast-multiply (no partition broadcast needed)
                    ps_o = ps2b.tile([128, 4, D + 1], F32, tag="ps_o")
                    for i in range(nq):
                        qt0 = qb + i
                        for kt in range(qt0 + 1):
                            c0 = off[kt] + qt0 * 128 - kt * 128
                            nc.tensor.matmul(
                                ps_o[:, i, :],
                                pt_sb[:, c0:c0 + 128],
                                vp[:, kt, h, :],
                                start=(kt == 0), stop=(kt == qt0))
                    dn = p2o.tile([128, 4], F32, tag="dn")
                    nc.vector.reciprocal(dn[:, 0:nq], ps_o[:, 0:nq, D])
                    nc.vector.tensor_tensor(
                        onrm[:, qb:qb + nq, hi, :],
                        ps_o[:, 0:nq, 0:D],
                        dn[:, 0:nq, None].to_broadcast([128, nq, D]),
                        OP.mult)
                if h % 2 == 0:
                    pending_tr = (onrm, ft)
            flush_tr()

        # ---------------- phase 3: projection ----------------
        with tc.tile_pool(name="p3", bufs=3) as p3, \
             tc.tile_pool(name="ps3", bufs=3, space="PSUM") as ps3:
            for t in range(NT):
                ps_y = ps3.tile([128, C], F32, tag="ps_y")
                for (n0, n1) in _chunks_512(0, C):
                    for kt in range(NC):
                        nc.tensor.matmul(
                            ps_y[:, n0:n1], attnT[:, kt, t * 128:(t + 1) * 128],
                            wpp[:, kt, n0:n1],
                            start=(kt == 0), stop=(kt == NC - 1))
                y_sb = p3.tile([128, C], F32, tag="y_sb")
                nc.vector.tensor_tensor(y_sb[:], ps_y[:],
                                        bb_sb[:, C:2 * C], OP.add)
                nc.sync.dma_start(io["y"][t * 128:(t + 1) * 128, :], y_sb[:])


IO_SPECS = {
    "xT": ([C, T], BF16),
    "wqkv": ([C, 3 * C], BF16),
    "bqkvT": ([128, 3 * C // 128], F32),
    "bias_bcast": ([128, 2 * C], F32),
    "wp": ([C, C], BF16),
    "ones": ([128, 128], F32),
    "negmaskT16": ([128, 128], BF16),
    "identity16": ([128, 128], BF16),
}
OUT_SPECS = {"y": ([T, C], F32)}


def build_nc():
    from concourse import bacc
    import concourse.tile as tile
    nc = bacc.Bacc("TRN2", target_bir_lowering=False, debug=False,
                   enable_asserts=True, num_devices=N_CORES)
    io = {}
    for name, (shape, dt) in IO_SPECS.items():
        io[name] = nc.dram_tensor(name, shape, dt, kind="ExternalInput").ap()
    for name, (shape, dt) in OUT_SPECS.items():
        io[name] = nc.dram_tensor(name, shape, dt, kind="ExternalOutput").ap()
    with tile.TileContext(nc) as tc:
        _emit_attention(tc, io)
    nc.compile()
    return nc


def host_consts():
    import ml_dtypes
    negmask = np.where(np.triu(np.ones((128, 128), dtype=bool)), 0.0,
                       -1e9).astype(np.float32)
    return {
        "ones": np.ones((128, 128), dtype=np.float32),
        "negmaskT16": np.ascontiguousarray(negmask.T).astype(ml_dtypes.bfloat16),
        "identity16": np.eye(128, dtype=ml_dtypes.bfloat16),
    }


_NC_CACHE = None


def _get_nc():
    global _NC_CACHE
    if _NC_CACHE is None:
        _NC_CACHE = build_nc()
    return _NC_CACHE


def make_in_maps(x, c_attn_kernel, c_attn_bias, c_proj_kernel, c_proj_bias):
    import ml_dtypes
    BF = ml_dtypes.bfloat16
    consts = host_consts()
    wqkv = np.ascontiguousarray(c_attn_kernel).astype(BF)
    bqkv = np.ascontiguousarray(c_attn_bias, dtype=np.float32)
    bqkvT = np.ascontiguousarray(bqkv.reshape(3 * C // 128, 128).T)
    wp = np.ascontiguousarray(c_proj_kernel).astype(BF)
    bp = np.ascontiguousarray(c_proj_bias, dtype=np.float32)
    bias_bcast = np.ascontiguousarray(
        np.tile(np.concatenate([bqkv[2 * C:], bp]), (128, 1)))
    in_maps = []
    for bb in range(N_CORES):
        m = {"xT": np.ascontiguousarray(np.asarray(x[bb]).T).astype(BF),
             "wqkv": wqkv, "bqkvT": bqkvT, "wp": wp,
             "bias_bcast": bias_bcast}
        m.update(consts)
        in_maps.append(m)
    return in_maps


def kernel(x, c_attn_kernel, c_attn_bias, c_proj_kernel, c_proj_bias):
    from concourse.bass_utils import run_bass_kernel_spmd
    x = np.asarray(x)
    assert x.shape == (B, T, C), x.shape
    nc = _get_nc()
    in_maps = make_in_maps(x, c_attn_kernel, c_attn_bias, c_proj_kernel,
                           c_proj_bias)
    res = run_bass_kernel_spmd(nc, in_maps, core_ids=list(range(N_CORES)))
    y = np.stack([res.results[bb]["y"] for bb in range(N_CORES)]).astype(np.float32)
    return y


# revision 26
# speedup vs baseline: 1.6398x; 1.4173x over previous
"""Causal self-attention (B=8, T=1024, C=768, H=12) on 8 Trainium2 NeuronCores.

Sharding: data parallel — one batch element per core, no collectives.

Per-core Bass/Tile kernel, all matmul operands bf16 (1 cyc/row at any N;
fp32 PSUM accumulate; host pre-quantizes weights + x to bf16):
  Q^T, K^T = Wqkv.T @ x^T                  (features on partitions)
  V = x^T.T @ Wv                           (natural layout, per-head + ones col)
  per head: S^T = K_h^T.T @ Q_h^T          (k on partitions, exact causal chunks)
    causal mask on the diagonal 128x128 block added on PSUM via a bf16
    matmul (negmask^T.T @ I), then ACT exp(s/8) -> ragged P^T
    O'^T = [V_h | 1].T @ P^T               (row 64 = softmax denominator)
    normalize: DVE reciprocal + tiny DMA to partition 0 + GPSIMD
    partition_broadcast + DVE multiply (odd heads take an SBUF->SBUF DMA to
    reach partitions 64..127 — DVE lanes are partition-locked)
  y = attn'^T.T @ Wp + bias
"""
import sys
from contextlib import ExitStack

import numpy as np

for _p in ("/opt/trn_rl_repo", "/root/.axon_site/_ro/trn_rl_repo"):
    if _p not in sys.path:
        sys.path.insert(0, _p)

import concourse.bass as bass  # noqa: E402
import concourse.mybir as mybir  # noqa: E402

F32 = mybir.dt.float32
BF16 = mybir.dt.bfloat16
AF = mybir.ActivationFunctionType
OP = mybir.AluOpType

B, T, C, H, D = 8, 1024, 768, 12, 64
N_CORES = 8


def _chunks_512(a, b):
    out = []
    while a < b:
        nxt = min((a // 512 + 1) * 512, b)
        out.append((a, nxt))
        a = nxt
    return out


def _emit_attention(tc, io):
    nc = tc.nc
    NT = T // 128
    NC = C // 128

    off = [0] * (NT + 1)
    for i in range(NT):
        off[i + 1] = off[i] + (T - 128 * i)
    PTW = off[NT]

    with ExitStack() as stack:
        persist = stack.enter_context(tc.tile_pool(name="persist", bufs=1))
        consts = stack.enter_context(tc.tile_pool(name="consts", bufs=1))
        negmT = consts.tile([128, 128], BF16, tag="negmT")
        idb = consts.tile([128, 128], BF16, tag="idb")
        nc.sync.dma_start(negmT[:], io["negmaskT16"])
        nc.sync.dma_start(idb[:], io["identity16"])
        bqt_sb = consts.tile([128, 3 * C // 128], F32, tag="bqt")
        nc.sync.dma_start(bqt_sb[:], io["bqkvT"])
        bb_sb = consts.tile([128, 2 * C], F32, tag="bb")
        nc.sync.dma_start(bb_sb[:], io["bias_bcast"])
        ones_sb = consts.tile([128, 128], F32, tag="ones_sb")
        nc.sync.dma_start(ones_sb[:], io["ones"])

        qt = persist.tile([128, NC, T], BF16, tag="qt")
        kt_ = persist.tile([128, NC, T], BF16, tag="kt")
        vp = persist.tile([128, NT, H, D + 1], BF16, tag="vp")
        attnT = persist.tile([128, NC, T], BF16, tag="attnT")
        wpp = persist.tile([128, NC, C], BF16, tag="wpp")
        x1t = persist.tile([128, NC, T], BF16, tag="x1t")
        wq_sb = persist.tile([128, NC, 3 * C], BF16, tag="wq")

        psu = stack.enter_context(tc.tile_pool(name="psu", bufs=3, space="PSUM"))
        ps2b = stack.enter_context(tc.tile_pool(name="ps2b", bufs=2, space="PSUM"))
        p2 = stack.enter_context(tc.tile_pool(name="p2", bufs=2))
        p2o = stack.enter_context(tc.tile_pool(name="p2o", bufs=3))
        p2p = stack.enter_context(tc.tile_pool(name="p2p", bufs=2))

        nc.vector.tensor_copy(
            vp[:, :, :, 64],
            ones_sb[:, 0:NT * H].rearrange("p (t h) -> p t h", h=H))

        # DMA order = PE consumption order: group ft consumes Q cols
        # [ft*128,(ft+1)*128), K cols C+same, V cols 2C+same. x is needed
        # in full by the very first matmul.
        def load_wq(w0, w1):
            for kt in range(NC):
                nc.sync.dma_start(
                    wq_sb[:, kt, w0:w1],
                    io["wqkv"][kt * 128:(kt + 1) * 128, w0:w1])

        def vcol(ft):
            return (2 * C + ft * 128, 2 * C + (ft + 1) * 128)

        load_wq(0, 256)
        load_wq(C, C + 256)
        load_wq(*vcol(0))
        for c in range(NC):
            nc.sync.dma_start(x1t[:, c, :],
                              io["xT"][c * 128:(c + 1) * 128, :])
        load_wq(*vcol(1))
        for mp in (1, 2):
            load_wq(mp * 256, mp * 256 + 256)
            load_wq(C + mp * 256, C + mp * 256 + 256)
            load_wq(*vcol(2 * mp))
            load_wq(*vcol(2 * mp + 1))
        for kt in range(NC):
            nc.sync.dma_start(wpp[:, kt, :],
                              io["wp"][kt * 128:(kt + 1) * 128, :])

        onrm = None
        pending_tr = None

        def flush_tr():
            nonlocal pending_tr
            if pending_tr is None:
                return
            onrm_p, ft_p = pending_tr
            pending_tr = None
            # transpose [q, (pair, d)] -> [(pair, d), q] feature-major
            ps_t = psu.tile([128, T], F32, tag="ps")
            ps_tb = ps_t.bitcast(BF16)
            for qt_ in range(NT):
                nc.tensor.transpose(ps_tb[:, qt_ * 128:(qt_ + 1) * 128],
                                    onrm_p[:, qt_, :, :], idb[:])
            nc.vector.tensor_copy(attnT[:, ft_p, :], ps_tb[:, 0:T])

        # ---- one group per feature tile: Q, K, V then its two heads ----
        for ft in range(NC):
            for m in (ft, NC + ft):
                dest = qt if m < NC else kt_
                ps = psu.tile([128, T], F32, tag="ps")
                for (a, b) in _chunks_512(0, T):
                    for kt in range(NC):
                        nc.tensor.matmul(
                            ps[:, a:b], wq_sb[:, kt, m * 128:(m + 1) * 128],
                            x1t[:, kt, a:b], start=(kt == 0),
                            stop=(kt == NC - 1))
                # bias is per-partition here: fuse it into the copy
                nc.vector.tensor_scalar_add(dest[:, ft, :], ps[:],
                                            bqt_sb[:, m:m + 1])

            # V columns for this head pair, all t tiles in one PSUM tile
            w0, w1 = vcol(ft)
            ps = psu.tile([128, T], F32, tag="ps")
            for t in range(NT):
                for kt in range(NC):
                    nc.tensor.matmul(
                        ps[:, t * 128:(t + 1) * 128],
                        x1t[:, kt, t * 128:(t + 1) * 128],
                        wq_sb[:, kt, w0:w1],
                        start=(kt == 0), stop=(kt == NC - 1))
            nc.vector.tensor_tensor(
                vp[:, :, 2 * ft:2 * ft + 2, 0:D],
                ps[:, 0:T].rearrange("p (t h d) -> p t h d", h=2, d=D),
                bb_sb[:, ft * 128:(ft + 1) * 128]
                .rearrange("p (h d) -> p h d", d=D)[:, None, :, :]
                .to_broadcast([128, NT, 2, D]),
                OP.add)

            for h in (2 * ft + 1, 2 * ft):
                p0 = 64 * (h % 2)      # partition base of this head's features
                pt_sb = p2.tile([128, PTW], BF16, tag="pt", name=f"pt{h}")
                for kt in range(NT):
                    base = (kt * 128 // 512) * 512
                    ps_s = psu.tile([128, T], F32, tag="ps")
                    for (a, b) in _chunks_512(kt * 128, T):
                        diag = a == kt * 128
                        nc.tensor.matmul(
                            ps_s[:, a - base:b - base],
                            kt_[p0:p0 + 64, ft, kt * 128:(kt + 1) * 128],
                            qt[p0:p0 + 64, ft, a:b],
                            start=True, stop=not diag)
                        if diag:
                            nc.tensor.matmul(ps_s[:, a - base:a - base + 128],
                                             negmT[:], idb[:],
                                             start=False, stop=True)
                    # one exp per strip -> ragged P^T
                    nc.scalar.activation(
                        pt_sb[:, off[kt]:off[kt + 1]],
                        ps_s[:, kt * 128 - base:T - base],
                        AF.Exp, bias=0.0, scale=1.0 / np.sqrt(D))
                if h % 2 == 1:
                    # first head of the feature-tile pair; previous pair's
                    # transposes flush here so they don't stall on its DVE
                    flush_tr()
                    onrm = p2p.tile([128, NT, 2, D], BF16, tag="onrm")
                hi = h % 2  # pair row: even head -> 0, odd -> 1
                for (q0, q1) in _chunks_512(0, T):
                    nq = (q1 - q0) // 128
                    qb = q0 // 128
                    # O = P^T.T @ [V|1]: q on partitions -> per-partition
                    # denominator in column D, normalize with a DVE
                    # broadcast-multiply (no partition broadcast needed)
                    ps_o = ps2b.tile([128, 4, D + 1], F32, tag="ps_o")
                    for i in range(nq):
                        qt0 = qb + i
                        for kt in range(qt0 + 1):
                            c0 = off[kt] + qt0 * 128 - kt * 128
                            nc.tensor.matmul(
                                ps_o[:, i, :],
                                pt_sb[:, c0:c0 + 128],
                                vp[:, kt, h, :],
                                start=(kt == 0), stop=(kt == qt0))
                    dn = p2o.tile([128, 4], F32, tag="dn")
                    nc.vector.reciprocal(dn[:, 0:nq], ps_o[:, 0:nq, D])
                    nc.vector.tensor_tensor(
                        onrm[:, qb:qb + nq, hi, :],
                        ps_o[:, 0:nq, 0:D],
                        dn[:, 0:nq, None].to_broadcast([128, nq, D]),
                        OP.mult)
                if h % 2 == 0:
                    pending_tr = (onrm, ft)
        flush_tr()

        # ---------------- phase 3: projection ----------------
        with tc.tile_pool(name="p3", bufs=3) as p3:
            for t in range(NT):
                ps_y = psu.tile([128, T], F32, tag="ps")
                for (n0, n1) in _chunks_512(0, C):
                    for kt in range(NC):
                        nc.tensor.matmul(
                            ps_y[:, n0:n1], attnT[:, kt, t * 128:(t + 1) * 128],
                            wpp[:, kt, n0:n1],
                            start=(kt == 0), stop=(kt == NC - 1))
                y_sb = p3.tile([128, C], F32, tag="y_sb")
                nc.vector.tensor_tensor(y_sb[:], ps_y[:, 0:C],
                                        bb_sb[:, C:2 * C], OP.add)
                nc.sync.dma_start(io["y"][t * 128:(t + 1) * 128, :], y_sb[:])


IO_SPECS = {
    "xT": ([C, T], BF16),
    "wqkv": ([C, 3 * C], BF16),
    "bqkvT": ([128, 3 * C // 128], F32),
    "bias_bcast": ([128, 2 * C], F32),
    "wp": ([C, C], BF16),
    "ones": ([128, 128], F32),
    "negmaskT16": ([128, 128], BF16),
    "identity16": ([128, 128], BF16),
}
OUT_SPECS = {"y": ([T, C], F32)}


def build_nc():
    from concourse import bacc
    import concourse.tile as tile
    nc = bacc.Bacc("TRN2", target_bir_lowering=False, debug=False,
                   enable_asserts=True, num_devices=N_CORES)
    io = {}
    for name, (shape, dt) in IO_SPECS.items():
        io[name] = nc.dram_tensor(name, shape, dt, kind="ExternalInput").ap()
    for name, (shape, dt) in OUT_SPECS.items():
        io[name] = nc.dram_tensor(name, shape, dt, kind="ExternalOutput").ap()
    with tile.TileContext(nc) as tc:
        _emit_attention(tc, io)
    nc.compile()
    return nc


def host_consts():
    import ml_dtypes
    negmask = np.where(np.triu(np.ones((128, 128), dtype=bool)), 0.0,
                       -1e9).astype(np.float32)
    return {
        "ones": np.ones((128, 128), dtype=np.float32),
        "negmaskT16": np.ascontiguousarray(negmask.T).astype(ml_dtypes.bfloat16),
        "identity16": np.eye(128, dtype=ml_dtypes.bfloat16),
    }


_NC_CACHE = None


def _get_nc():
    global _NC_CACHE
    if _NC_CACHE is None:
        _NC_CACHE = build_nc()
    return _NC_CACHE


def make_in_maps(x, c_attn_kernel, c_attn_bias, c_proj_kernel, c_proj_bias):
    import ml_dtypes
    BF = ml_dtypes.bfloat16
    consts = host_consts()
    wqkv = np.ascontiguousarray(c_attn_kernel).astype(BF)
    bqkv = np.ascontiguousarray(c_attn_bias, dtype=np.float32)
    bqkvT = np.ascontiguousarray(bqkv.reshape(3 * C // 128, 128).T)
    wp = np.ascontiguousarray(c_proj_kernel).astype(BF)
    bp = np.ascontiguousarray(c_proj_bias, dtype=np.float32)
    bias_bcast = np.ascontiguousarray(
        np.tile(np.concatenate([bqkv[2 * C:], bp]), (128, 1)))
    in_maps = []
    for bb in range(N_CORES):
        m = {"xT": np.ascontiguousarray(np.asarray(x[bb]).T).astype(BF),
             "wqkv": wqkv, "bqkvT": bqkvT, "wp": wp,
             "bias_bcast": bias_bcast}
        m.update(consts)
        in_maps.append(m)
    return in_maps


def kernel(x, c_attn_kernel, c_attn_bias, c_proj_kernel, c_proj_bias):
    from concourse.bass_utils import run_bass_kernel_spmd
    x = np.asarray(x)
    assert x.shape == (B, T, C), x.shape
    nc = _get_nc()
    in_maps = make_in_maps(x, c_attn_kernel, c_attn_bias, c_proj_kernel,
                           c_proj_bias)
    res = run_bass_kernel_spmd(nc, in_maps, core_ids=list(range(N_CORES)))
    y = np.stack([res.results[bb]["y"] for bb in range(N_CORES)]).astype(np.float32)
    return y
